# revision 12
# baseline (speedup 1.0000x reference)
"""Two-NEFF Trainium2 kernel for fused BatchNorm1d(train) -> Linear -> ELU.

  y = ELU( ((x - mean) * gamma.rsqrt(var+eps) + beta) @ W.T )

Data-parallel over 8 cores (rows sharded). BN stats are reduced on the HOST
between two NEFF launches (a 4 KB exchange; an on-device collective measured
~0.5 ms slower in a previous session).

Key layout decision vs the earlier baseline: the earlier kernel staged x in
ROW-major bf16 and phase C re-read it with `dma_start_transpose`, which
measured ~106 GB/s effective (xbar-transpose DMA serializes badly) and made
phase C 1.26 ms. Here phase A transposes ON-CHIP with the PE (identity
matmul) and stages x TRANSPOSED, so phase C does only plain contiguous DMA.

  NEFF A (per core): stream x tiles (bf16, host pre-cast), PE-transpose
      16x [128,128] blocks per tile into PSUM, ACT-copy to bf16 SBUF
      (accum_out gives per-feature sums for free), DVE square-reduce gives
      per-feature sum-of-squares; stage transposed tiles to DRAM.
      Stats out: st [128, 4] = (sum_h0, sum_h1, ssq_h0, ssq_h1).
  host: sum the 8 st tiles, finalize scale/shift s,t in f64:
      s = gamma * rsqrt(var+eps), t = beta - mean*s.
  NEFF C (per core): preamble folds s into W.T (bf16) and computes the
      bias row b = t @ W.T with two rank-128 matmuls; main loop reads
      staged xT tiles (contiguous), 3 matmuls per 128-row group
      (two f_in halves + rank-1 bias), ELU = min(exp(y)-1, relu(y)),
      writes y in bf16 (upcast to f32 on host).

Row mapping: x rows are loaded as [t, p, j] (row = t*1024 + p*8 + j), the
j-th block-column of the transposed tile holds rows {c*8+j}; after the
matmul, PSUM partition q of group j is row q*8+j, so y tiles write back
8 CONSECUTIVE rows per partition (4 KiB contiguous descriptors).
"""

import functools
import sys

import numpy as np

if "/opt/trn_rl_repo" not in sys.path:
    sys.path.insert(0, "/opt/trn_rl_repo")

N_TOTAL = 1048576
F = 256
NCORES = 8
N_SHARD = N_TOTAL // NCORES
P = 128
RT = 8
T = N_SHARD // (P * RT)
EPS = 1e-5


def _bass(ncores):
    from concourse import bacc

    return bacc.Bacc(
        "TRN2", target_bir_lowering=False, debug=False, num_devices=ncores
    )


def build_a(n_shard=N_SHARD, ncores=NCORES, repeat=1):
    """Phase A: on-chip transpose to bf16 staging + BN partial stats.

    Inputs: x [n_shard, 256] bf16, ident [128, 128] bf16.
    Outputs: xt [(T*2*128), 1024] bf16 (transposed staging),
             st [128, 4] f32 = (sum_h0, sum_h1, ssq_h0, ssq_h1).
    """
    import concourse.tile as tile
    from concourse import mybir

    f32 = mybir.dt.float32
    bf16 = mybir.dt.bfloat16
    AF = mybir.ActivationFunctionType
    OP = mybir.AluOpType
    AX = mybir.AxisListType

    t_count = n_shard // (P * RT)

    nc = _bass(ncores)
    x = nc.dram_tensor("x", [n_shard, F], bf16, kind="ExternalInput").ap()
    ident = nc.dram_tensor("ident", [P, P], bf16, kind="ExternalInput").ap()
    xt = nc.dram_tensor(
        "xt", [t_count * 2 * P, RT * P], bf16, kind="ExternalOutput"
    ).ap()
    st = nc.dram_tensor("st", [P, 4], f32, kind="ExternalOutput").ap()

    with tile.TileContext(nc) as tc:
        with tc.tile_pool(name="wp", bufs=1) as wp:
            id_sb = wp.tile([P, P], bf16)
            nc.sync.dma_start(id_sb[:], ident)
            for _rep in range(repeat):
                with tc.tile_pool(name="sa", bufs=4) as sa, tc.tile_pool(
                    name="sbp", bufs=1
                ) as sbp, tc.tile_pool(name="psA", bufs=8, space="PSUM") as psA:
                    sum_buf = sbp.tile([P, 2, t_count], f32)
                    ssq_buf = sbp.tile([P, 2, t_count], f32)
                    xv = x.rearrange("(t p j) f -> t p j f", p=P, j=RT)
                    xtv = xt.rearrange("(t h p) c -> t h p c", h=2, p=P)
                    for t in range(t_count):
                        xin = sa.tile([P, RT, F], bf16, tag="xin")
                        nc.sync.dma_start(xin[:], xv[t])
                        for h in range(2):
                            # per-h psum tiles (1 bank, 8 bufs): finer release
                            # granularity lets the PE run further ahead
                            psh = psA.tile([P, RT * P], bf16, tag="psh")
                            for j in range(RT):
                                nc.tensor.transpose(
                                    psh[:, j * P : (j + 1) * P],
                                    xin[:, j, h * P : (h + 1) * P],
                                    id_sb[:],
                                )
                            xth = sa.tile([P, RT * P], bf16, tag=f"xt{h}")
                            # PSUM bf16 -> SBUF bf16; accum_out = per-feature sum
                            nc.scalar.activation(
                                xth[:],
                                psh[:],
                                AF.Identity,
                                accum_out=sum_buf[:, h, t : t + 1],
                            )
                            nc.sync.dma_start(xtv[t, h], xth[:])
                            # DVE accum_out crashes on HW here; square via
                            # plain tensor_tensor, then free-axis reduce.
                            scr = sa.tile([P, RT * P], bf16, tag=f"scr{h}")
                            nc.vector.tensor_tensor(
                                scr[:], xth[:], xth[:], OP.mult
                            )
                            nc.vector.tensor_reduce(
                                ssq_buf[:, h, t : t + 1], scr[:], AX.X, OP.add
                            )
                    stv = sbp.tile([P, 4], f32)
                    for h in range(2):
                        nc.vector.tensor_reduce(
                            stv[:, h : h + 1], sum_buf[:, h], AX.X, OP.add
                        )
                        nc.vector.tensor_reduce(
                            stv[:, 2 + h : 3 + h], ssq_buf[:, h], AX.X, OP.add
                        )
                    nc.sync.dma_start(st, stv[:])
    nc.compile()
    return nc


def build_c(n_shard=N_SHARD, ncores=NCORES, repeat=1):
    """Phase C: matmul from transposed staging + ELU, TRANSPOSED bf16 output.

    Computes yT = (s*W.T).T-blocks @ xT + b so the small W blocks are the
    PE-stationary operand (4 reused loads per tile instead of 16) and the
    linear bias b = t @ W.T is PER-PARTITION, riding the ACT/DVE ops for
    free. The host un-transposes the blocked output.

    Inputs: xt [(T*2*128), 1024] bf16, wt [256, 256] f32 (= W.T),
            aff [128, 4] f32 = (s_h0, s_h1, b_q0, b_q1).
    Output: yt [(T*2*128), 1024] bf16, blocked [t, q, p_fout, (j, c)]
            = y[row t*1024 + c*8 + j, fout q*128 + p_fout].
    """
    import concourse.tile as tile
    from concourse import mybir

    f32 = mybir.dt.float32
    bf16 = mybir.dt.bfloat16
    AF = mybir.ActivationFunctionType
    OP = mybir.AluOpType

    t_count = n_shard // (P * RT)
    NB = RT * P // 2  # 512: psum-bank-sized matmul N

    nc = _bass(ncores)
    xt = nc.dram_tensor(
        "xt", [t_count * 2 * P, RT * P], bf16, kind="ExternalInput"
    ).ap()
    wt = nc.dram_tensor("wt", [F, F], f32, kind="ExternalInput").ap()
    aff = nc.dram_tensor("aff", [P, 4], f32, kind="ExternalInput").ap()
    yt = nc.dram_tensor(
        "yt", [t_count * 2 * P, RT * P], bf16, kind="ExternalOutput"
    ).ap()

    with tile.TileContext(nc) as tc:
        with tc.tile_pool(name="wp", bufs=1) as wp:
            for _rep in range(repeat):
                with tc.tile_pool(name="pre", bufs=1) as pre:
                    wt_sb = pre.tile([P, 2, F], f32)
                    nc.sync.dma_start(
                        wt_sb[:], wt.rearrange("(h p) f -> p h f", p=P)
                    )
                    aff_sb = wp.tile([P, 4], f32)
                    nc.sync.dma_start(aff_sb[:], aff)
                    # ws[h] = W.T[h-half] * s[h] (bf16)
                    ws = wp.tile([P, 2, F], bf16)
                    for h in range(2):
                        nc.vector.tensor_scalar(
                            ws[:, h],
                            wt_sb[:, h],
                            aff_sb[:, h : h + 1],
                            None,
                            OP.mult,
                        )

                with tc.tile_pool(name="cp", bufs=4) as cp, tc.tile_pool(
                    name="psC", bufs=4, space="PSUM"
                ) as psC:
                    xtv = xt.rearrange("(t h p) c -> t h p c", h=2, p=P)
                    ytv = yt.rearrange("(t q p) c -> t q p c", q=2, p=P)
                    for t in range(t_count):
                        xt0 = cp.tile([P, RT * P], bf16, tag="x0")
                        nc.sync.dma_start(xt0[:], xtv[t, 0])
                        xt1 = cp.tile([P, RT * P], bf16, tag="x1")
                        nc.sync.dma_start(xt1[:], xtv[t, 1])
                        xth = [xt0, xt1]
                        # per-q psum tiles (2 banks each, 4 bufs) let the PE
                        # run ahead while ELU drains earlier groups
                        for q in range(2):
                            ps = psC.tile([P, 2, NB], f32, tag="psy")
                            for h in range(2):
                                wblk = ws[:, h, q * P : (q + 1) * P]
                                for n in range(2):
                                    nc.tensor.matmul(
                                        ps[:, n],
                                        wblk,
                                        xth[h][:, n * NB : (n + 1) * NB],
                                        start=(h == 0),
                                        stop=(h == 1),
                                    )
                            # ELU(v+b) = min(exp(v+b)-1, relu(v+b)), b per-part
                            bcol = aff_sb[:, 2 + q : 3 + q]
                            e = cp.tile([P, 2 * NB], bf16, tag=f"e{q}")
                            nc.scalar.activation(
                                e[:], ps[:], AF.Exp, bias=bcol
                            )
                            r = cp.tile([P, 2 * NB], bf16, tag=f"r{q}")
                            if q == 0:
                                nc.scalar.activation(
                                    r[:], ps[:], AF.Relu, bias=bcol
                                )
                            else:
                                nc.vector.tensor_scalar(
                                    r[:], ps[:], bcol, 0.0, OP.add, OP.max
                                )
                            yo = cp.tile([P, 2 * NB], bf16, tag=f"yo{q}")
                            nc.vector.scalar_tensor_tensor(
                                yo[:], e[:], 1.0, r[:], OP.subtract, OP.min
                            )
                            nc.sync.dma_start(ytv[t, q], yo[:])
    nc.compile()
    return nc


@functools.lru_cache(maxsize=4)
def _built_a(repeat=1):
    return build_a(repeat=repeat)


@functools.lru_cache(maxsize=4)
def _built_c(repeat=1):
    return build_c(repeat=repeat)


def _pjrt_fn(nc, ncores=NCORES):
    """Compile a bass module into a jitted 8-core shard_map callable.
    Returns (fn, in_names, out_names, out_avals, mesh)."""
    import jax
    from jax.experimental.shard_map import shard_map
    from jax.sharding import Mesh, PartitionSpec

    from concourse import mybir
    from concourse.bass2jax import (
        _bass_exec_p,
        install_neuronx_cc_hook,
        partition_id_tensor,
    )

    install_neuronx_cc_hook()
    partition_name = nc.partition_id_tensor.name if nc.partition_id_tensor else None
    in_names, out_names, out_avals = [], [], []
    for alloc in nc.m.functions[0].allocations:
        if not isinstance(alloc, mybir.MemoryLocationSet):
            continue
        name = alloc.memorylocations[0].name
        if alloc.kind == "ExternalInput":
            if name != partition_name:
                in_names.append(name)
        elif alloc.kind == "ExternalOutput":
            out_names.append(name)
            out_avals.append(
                jax.core.ShapedArray(
                    tuple(alloc.tensor_shape), mybir.dt.np(alloc.dtype)
                )
            )
    n_params = len(in_names)
    all_in_names = list(in_names) + list(out_names)
    if partition_name is not None:
        all_in_names.append(partition_name)

    def _body(*args):
        operands = list(args)
        if partition_name is not None:
            operands.append(partition_id_tensor())
        outs = _bass_exec_p.bind(
            *operands,
            out_avals=tuple(out_avals),
            in_names=tuple(all_in_names),
            out_names=tuple(out_names),
            lowering_input_output_aliases=(),
            sim_require_finite=True,
            sim_require_nnan=True,
            nc=nc,
        )
        return tuple(outs)

    devices = jax.devices()[:ncores]
    mesh = Mesh(np.asarray(devices), ("core",))
    spec = PartitionSpec("core")
    fn = jax.jit(
        shard_map(
            _body,
            mesh=mesh,
            in_specs=(spec,) * (n_params + len(out_names)),
            out_specs=(spec,) * len(out_names),
            check_rep=False,
        ),
        keep_unused=True,
    )
    return fn, in_names, out_names, out_avals, mesh


def _sharding():
    import jax
    from jax.sharding import Mesh, NamedSharding, PartitionSpec

    devices = jax.devices()[:NCORES]
    mesh = Mesh(np.asarray(devices), ("core",))
    return NamedSharding(mesh, PartitionSpec("core"))


def _zeros_for(out_avals):
    return [
        np.zeros((NCORES * av.shape[0], *av.shape[1:]), av.dtype) for av in out_avals
    ]


def kernel(x, gamma, beta, W):
    import jax
    import jax.numpy as jnp

    gamma = np.asarray(gamma, dtype=np.float64)
    beta = np.asarray(beta, dtype=np.float64)
    W = np.asarray(W, dtype=np.float32)
    assert np.asarray(x).shape == (N_TOTAL, F)

    cpu = jax.devices("cpu")[0]
    with jax.default_device(cpu):
        x_bf = np.asarray(jnp.asarray(np.asarray(x)).astype(jnp.bfloat16))

    sharding = _sharding()

    # ---- NEFF A: on-chip transpose + staging + partial stats
    nc_a = _built_a()
    fn_a, in_a, out_a, av_a, _ = _pjrt_fn(nc_a)
    ident = np.concatenate([np.eye(P, dtype=x_bf.dtype)] * NCORES, axis=0)
    host_a = {"x": x_bf, "ident": ident}
    args_a = [jax.device_put(host_a[nm], sharding) for nm in in_a]
    outs_a = fn_a(*args_a, *[jax.device_put(z, sharding) for z in _zeros_for(av_a)])
    outs_a = dict(zip(out_a, outs_a))

    # ---- host: reduce the 8 partial stat tiles (16 KB), finalize scale/shift
    st_host = np.asarray(outs_a["st"]).astype(np.float64)  # [8*128, 4]
    st_sum = st_host.reshape(NCORES, P, 4).sum(axis=0)  # [128, 4]
    mean = st_sum[:, 0:2] / N_TOTAL  # [128, 2] (h columns)
    var = st_sum[:, 2:4] / N_TOTAL - mean**2
    g_cols = np.stack([gamma[0:P], gamma[P:F]], axis=1)
    b_cols = np.stack([beta[0:P], beta[P:F]], axis=1)
    s_cols = g_cols / np.sqrt(var + EPS)
    t_cols = b_cols - mean * s_cols
    # linear bias row b = t @ W.T, split into f_out halves (per-partition on C)
    t_vec = np.concatenate([t_cols[:, 0], t_cols[:, 1]])
    b_row = t_vec @ W.astype(np.float64).T
    bq_cols = np.stack([b_row[0:P], b_row[P:F]], axis=1)
    aff = np.concatenate([s_cols, bq_cols], axis=1).astype(np.float32)  # [128,4]

    # ---- NEFF C: matmul + ELU (staging stays on device)
    nc_c = _built_c()
    fn_c, in_c, out_c, av_c, _ = _pjrt_fn(nc_c)
    host_c = {
        "wt": np.concatenate([np.ascontiguousarray(W.T)] * NCORES, axis=0),
        "aff": np.concatenate([aff] * NCORES, axis=0),
    }
    args_c = []
    for nm in in_c:
        if nm == "xt":
            args_c.append(outs_a["xt"])
        else:
            args_c.append(jax.device_put(host_c[nm], sharding))
    outs_c = fn_c(*args_c, *[jax.device_put(z, sharding) for z in _zeros_for(av_c)])
    y_bf = np.asarray(outs_c[out_c.index("yt")])
    with jax.default_device(cpu):
        # yt blocked [core, t, q, p, j, c] -> y[row t*1024+c*8+j, fout q*128+p]
        yt6 = jnp.asarray(y_bf).reshape(NCORES, T, 2, P, RT, P)
        y = np.asarray(
            jnp.transpose(yt6, (0, 1, 5, 4, 2, 3))
            .astype(jnp.float32)
            .reshape(N_TOTAL, F)
        )
    return np.ascontiguousarray(y)


if __name__ == "__main__":
    nca = build_a()
    ncc = build_c()
    print("built OK")


# revision 13
# speedup vs baseline: 1.0966x; 1.0966x over previous
"""Two-NEFF Trainium2 kernel for fused BatchNorm1d(train) -> Linear -> ELU.

  y = ELU( ((x - mean) * gamma.rsqrt(var+eps) + beta) @ W.T )

Data-parallel over 8 cores (rows sharded). BN stats are reduced on the HOST
between two NEFF launches (a 4 KB exchange; an on-device collective measured
~0.5 ms slower in a previous session).

Key layout decision vs the earlier baseline: the earlier kernel staged x in
ROW-major bf16 and phase C re-read it with `dma_start_transpose`, which
measured ~106 GB/s effective (xbar-transpose DMA serializes badly) and made
phase C 1.26 ms. Here phase A transposes ON-CHIP with the PE (identity
matmul) and stages x TRANSPOSED, so phase C does only plain contiguous DMA.

  NEFF A (per core): stream x tiles (bf16, host pre-cast), PE-transpose
      16x [128,128] blocks per tile into PSUM, ACT-copy to bf16 SBUF
      (accum_out gives per-feature sums for free), DVE square-reduce gives
      per-feature sum-of-squares; stage transposed tiles to DRAM.
      Stats out: st [128, 4] = (sum_h0, sum_h1, ssq_h0, ssq_h1).
  host: sum the 8 st tiles, finalize scale/shift s,t in f64:
      s = gamma * rsqrt(var+eps), t = beta - mean*s.
  NEFF C (per core): preamble folds s into W.T (bf16) and computes the
      bias row b = t @ W.T with two rank-128 matmuls; main loop reads
      staged xT tiles (contiguous), 3 matmuls per 128-row group
      (two f_in halves + rank-1 bias), ELU = min(exp(y)-1, relu(y)),
      writes y in bf16 (upcast to f32 on host).

Row mapping: x rows are loaded as [t, p, j] (row = t*1024 + p*8 + j), the
j-th block-column of the transposed tile holds rows {c*8+j}; after the
matmul, PSUM partition q of group j is row q*8+j, so y tiles write back
8 CONSECUTIVE rows per partition (4 KiB contiguous descriptors).
"""

import functools
import sys

import numpy as np

if "/opt/trn_rl_repo" not in sys.path:
    sys.path.insert(0, "/opt/trn_rl_repo")

N_TOTAL = 1048576
F = 256
NCORES = 8
N_SHARD = N_TOTAL // NCORES
P = 128
RT = 8
T = N_SHARD // (P * RT)
EPS = 1e-5


def _bass(ncores):
    from concourse import bacc

    return bacc.Bacc(
        "TRN2", target_bir_lowering=False, debug=False, num_devices=ncores
    )


def build_a(n_shard=N_SHARD, ncores=NCORES, repeat=1):
    """Phase A: on-chip transpose to bf16 staging + BN partial stats.

    Inputs: x [n_shard, 256] bf16, ident [128, 128] bf16.
    Outputs: xt [(T*2*128), 1024] bf16 (transposed staging),
             st [128, 4] f32 = (sum_h0, sum_h1, ssq_h0, ssq_h1).
    """
    import concourse.tile as tile
    from concourse import mybir

    f32 = mybir.dt.float32
    bf16 = mybir.dt.bfloat16
    AF = mybir.ActivationFunctionType
    OP = mybir.AluOpType
    AX = mybir.AxisListType

    t_count = n_shard // (P * RT)

    nc = _bass(ncores)
    x = nc.dram_tensor("x", [n_shard, F], bf16, kind="ExternalInput").ap()
    ident = nc.dram_tensor("ident", [P, P], bf16, kind="ExternalInput").ap()
    xt = nc.dram_tensor(
        "xt", [t_count * 2 * P, RT * P], bf16, kind="ExternalOutput"
    ).ap()
    st = nc.dram_tensor("st", [P, 4], f32, kind="ExternalOutput").ap()

    with tile.TileContext(nc) as tc:
        with tc.tile_pool(name="wp", bufs=1) as wp:
            id_sb = wp.tile([P, P], bf16)
            nc.sync.dma_start(id_sb[:], ident)
            for _rep in range(repeat):
                with tc.tile_pool(name="sa", bufs=4) as sa, tc.tile_pool(
                    name="sbp", bufs=1
                ) as sbp, tc.tile_pool(name="psA", bufs=4, space="PSUM") as psA:
                    sum_buf = sbp.tile([P, 2, t_count], f32)
                    ssq_buf = sbp.tile([P, 2, t_count], f32)
                    xv = x.rearrange("(t p j) f -> t p j f", p=P, j=RT)
                    xtv = xt.rearrange("(t h p) c -> t h p c", h=2, p=P)
                    for t in range(t_count):
                        xin = sa.tile([P, RT, F], bf16, tag="xin")
                        nc.sync.dma_start(xin[:], xv[t])
                        ps = psA.tile([P, 2, RT * P], bf16, tag="ps")
                        for h in range(2):
                            for j in range(RT):
                                nc.tensor.transpose(
                                    ps[:, h, j * P : (j + 1) * P],
                                    xin[:, j, h * P : (h + 1) * P],
                                    id_sb[:],
                                )
                        for h in range(2):
                            xth = sa.tile([P, RT * P], bf16, tag=f"xt{h}")
                            # PSUM f32 -> SBUF bf16; accum_out = per-feature sum
                            nc.scalar.activation(
                                xth[:],
                                ps[:, h],
                                AF.Identity,
                                accum_out=sum_buf[:, h, t : t + 1],
                            )
                            nc.sync.dma_start(xtv[t, h], xth[:])
                            # DVE accum_out crashes on HW here; square via
                            # plain tensor_tensor, then free-axis reduce.
                            scr = sa.tile([P, RT * P], bf16, tag=f"scr{h}")
                            nc.vector.tensor_tensor(
                                scr[:], xth[:], xth[:], OP.mult
                            )
                            nc.vector.tensor_reduce(
                                ssq_buf[:, h, t : t + 1], scr[:], AX.X, OP.add
                            )
                    stv = sbp.tile([P, 4], f32)
                    for h in range(2):
                        nc.vector.tensor_reduce(
                            stv[:, h : h + 1], sum_buf[:, h], AX.X, OP.add
                        )
                        nc.vector.tensor_reduce(
                            stv[:, 2 + h : 3 + h], ssq_buf[:, h], AX.X, OP.add
                        )
                    nc.sync.dma_start(st, stv[:])
    nc.compile()
    return nc


def build_c(n_shard=N_SHARD, ncores=NCORES, repeat=1):
    """Phase C: matmul from transposed staging + ELU, TRANSPOSED bf16 output.

    Computes yT = (s*W.T).T-blocks @ xT + b so the small W blocks are the
    PE-stationary operand (4 reused loads per tile instead of 16) and the
    linear bias b = t @ W.T is PER-PARTITION, riding the ACT/DVE ops for
    free. The host un-transposes the blocked output.

    Inputs: xt [(T*2*128), 1024] bf16, wt [256, 256] f32 (= W.T),
            aff [128, 4] f32 = (s_h0, s_h1, b_q0, b_q1).
    Output: yt [(T*2*128), 1024] bf16, blocked [t, q, p_fout, (j, c)]
            = y[row t*1024 + c*8 + j, fout q*128 + p_fout].
    """
    import concourse.tile as tile
    from concourse import mybir

    f32 = mybir.dt.float32
    bf16 = mybir.dt.bfloat16
    AF = mybir.ActivationFunctionType
    OP = mybir.AluOpType

    t_count = n_shard // (P * RT)
    NB = RT * P // 2  # 512: psum-bank-sized matmul N

    nc = _bass(ncores)
    xt = nc.dram_tensor(
        "xt", [t_count * 2 * P, RT * P], bf16, kind="ExternalInput"
    ).ap()
    wt = nc.dram_tensor("wt", [F, F], f32, kind="ExternalInput").ap()
    aff = nc.dram_tensor("aff", [P, 4], f32, kind="ExternalInput").ap()
    yt = nc.dram_tensor(
        "yt", [t_count * 2 * P, RT * P], bf16, kind="ExternalOutput"
    ).ap()

    with tile.TileContext(nc) as tc:
        with tc.tile_pool(name="wp", bufs=1) as wp:
            for _rep in range(repeat):
                with tc.tile_pool(name="pre", bufs=1) as pre:
                    wt_sb = pre.tile([P, 2, F], f32)
                    nc.sync.dma_start(
                        wt_sb[:], wt.rearrange("(h p) f -> p h f", p=P)
                    )
                    aff_sb = wp.tile([P, 4], f32)
                    nc.sync.dma_start(aff_sb[:], aff)
                    # ws[h] = W.T[h-half] * s[h] (bf16)
                    ws = wp.tile([P, 2, F], bf16)
                    for h in range(2):
                        nc.vector.tensor_scalar(
                            ws[:, h],
                            wt_sb[:, h],
                            aff_sb[:, h : h + 1],
                            None,
                            OP.mult,
                        )

                with tc.tile_pool(name="cp", bufs=4) as cp, tc.tile_pool(
                    name="psC", bufs=4, space="PSUM"
                ) as psC:
                    xtv = xt.rearrange("(t h p) c -> t h p c", h=2, p=P)
                    ytv = yt.rearrange("(t q p) c -> t q p c", q=2, p=P)
                    for t in range(t_count):
                        xt0 = cp.tile([P, RT * P], bf16, tag="x0")
                        nc.sync.dma_start(xt0[:], xtv[t, 0])
                        xt1 = cp.tile([P, RT * P], bf16, tag="x1")
                        nc.sync.dma_start(xt1[:], xtv[t, 1])
                        xth = [xt0, xt1]
                        # per-q psum tiles (2 banks each, 4 bufs) let the PE
                        # run ahead while ELU drains earlier groups
                        for q in range(2):
                            ps = psC.tile([P, 2, NB], f32, tag="psy")
                            for h in range(2):
                                wblk = ws[:, h, q * P : (q + 1) * P]
                                for n in range(2):
                                    nc.tensor.matmul(
                                        ps[:, n],
                                        wblk,
                                        xth[h][:, n * NB : (n + 1) * NB],
                                        start=(h == 0),
                                        stop=(h == 1),
                                    )
                            # ELU(v+b) = min(exp(v+b)-1, relu(v+b)), b per-part
                            bcol = aff_sb[:, 2 + q : 3 + q]
                            e = cp.tile([P, 2 * NB], bf16, tag=f"e{q}")
                            nc.scalar.activation(
                                e[:], ps[:], AF.Exp, bias=bcol
                            )
                            r = cp.tile([P, 2 * NB], bf16, tag=f"r{q}")
                            if q == 0:
                                nc.scalar.activation(
                                    r[:], ps[:], AF.Relu, bias=bcol
                                )
                            else:
                                nc.vector.tensor_scalar(
                                    r[:], ps[:], bcol, 0.0, OP.add, OP.max
                                )
                            yo = cp.tile([P, 2 * NB], bf16, tag=f"yo{q}")
                            nc.vector.scalar_tensor_tensor(
                                yo[:], e[:], 1.0, r[:], OP.subtract, OP.min
                            )
                            nc.sync.dma_start(ytv[t, q], yo[:])
    nc.compile()
    return nc


@functools.lru_cache(maxsize=4)
def _built_a(repeat=1):
    return build_a(repeat=repeat)


@functools.lru_cache(maxsize=4)
def _built_c(repeat=1):
    return build_c(repeat=repeat)


def _pjrt_fn(nc, ncores=NCORES):
    """Compile a bass module into a jitted 8-core shard_map callable.
    Returns (fn, in_names, out_names, out_avals, mesh)."""
    import jax
    from jax.experimental.shard_map import shard_map
    from jax.sharding import Mesh, PartitionSpec

    from concourse import mybir
    from concourse.bass2jax import (
        _bass_exec_p,
        install_neuronx_cc_hook,
        partition_id_tensor,
    )

    install_neuronx_cc_hook()
    partition_name = nc.partition_id_tensor.name if nc.partition_id_tensor else None
    in_names, out_names, out_avals = [], [], []
    for alloc in nc.m.functions[0].allocations:
        if not isinstance(alloc, mybir.MemoryLocationSet):
            continue
        name = alloc.memorylocations[0].name
        if alloc.kind == "ExternalInput":
            if name != partition_name:
                in_names.append(name)
        elif alloc.kind == "ExternalOutput":
            out_names.append(name)
            out_avals.append(
                jax.core.ShapedArray(
                    tuple(alloc.tensor_shape), mybir.dt.np(alloc.dtype)
                )
            )
    n_params = len(in_names)
    all_in_names = list(in_names) + list(out_names)
    if partition_name is not None:
        all_in_names.append(partition_name)

    def _body(*args):
        operands = list(args)
        if partition_name is not None:
            operands.append(partition_id_tensor())
        outs = _bass_exec_p.bind(
            *operands,
            out_avals=tuple(out_avals),
            in_names=tuple(all_in_names),
            out_names=tuple(out_names),
            lowering_input_output_aliases=(),
            sim_require_finite=True,
            sim_require_nnan=True,
            nc=nc,
        )
        return tuple(outs)

    devices = jax.devices()[:ncores]
    mesh = Mesh(np.asarray(devices), ("core",))
    spec = PartitionSpec("core")
    fn = jax.jit(
        shard_map(
            _body,
            mesh=mesh,
            in_specs=(spec,) * (n_params + len(out_names)),
            out_specs=(spec,) * len(out_names),
            check_rep=False,
        ),
        keep_unused=True,
    )
    return fn, in_names, out_names, out_avals, mesh


def _sharding():
    import jax
    from jax.sharding import Mesh, NamedSharding, PartitionSpec

    devices = jax.devices()[:NCORES]
    mesh = Mesh(np.asarray(devices), ("core",))
    return NamedSharding(mesh, PartitionSpec("core"))


def _zeros_for(out_avals):
    return [
        np.zeros((NCORES * av.shape[0], *av.shape[1:]), av.dtype) for av in out_avals
    ]


def kernel(x, gamma, beta, W):
    import jax
    import jax.numpy as jnp

    gamma = np.asarray(gamma, dtype=np.float64)
    beta = np.asarray(beta, dtype=np.float64)
    W = np.asarray(W, dtype=np.float32)
    assert np.asarray(x).shape == (N_TOTAL, F)

    cpu = jax.devices("cpu")[0]
    with jax.default_device(cpu):
        x_bf = np.asarray(jnp.asarray(np.asarray(x)).astype(jnp.bfloat16))

    sharding = _sharding()

    # ---- NEFF A: on-chip transpose + staging + partial stats
    nc_a = _built_a()
    fn_a, in_a, out_a, av_a, _ = _pjrt_fn(nc_a)
    ident = np.concatenate([np.eye(P, dtype=x_bf.dtype)] * NCORES, axis=0)
    host_a = {"x": x_bf, "ident": ident}
    args_a = [jax.device_put(host_a[nm], sharding) for nm in in_a]
    outs_a = fn_a(*args_a, *[jax.device_put(z, sharding) for z in _zeros_for(av_a)])
    outs_a = dict(zip(out_a, outs_a))

    # ---- host: reduce the 8 partial stat tiles (16 KB), finalize scale/shift
    st_host = np.asarray(outs_a["st"]).astype(np.float64)  # [8*128, 4]
    st_sum = st_host.reshape(NCORES, P, 4).sum(axis=0)  # [128, 4]
    mean = st_sum[:, 0:2] / N_TOTAL  # [128, 2] (h columns)
    var = st_sum[:, 2:4] / N_TOTAL - mean**2
    g_cols = np.stack([gamma[0:P], gamma[P:F]], axis=1)
    b_cols = np.stack([beta[0:P], beta[P:F]], axis=1)
    s_cols = g_cols / np.sqrt(var + EPS)
    t_cols = b_cols - mean * s_cols
    # linear bias row b = t @ W.T, split into f_out halves (per-partition on C)
    t_vec = np.concatenate([t_cols[:, 0], t_cols[:, 1]])
    b_row = t_vec @ W.astype(np.float64).T
    bq_cols = np.stack([b_row[0:P], b_row[P:F]], axis=1)
    aff = np.concatenate([s_cols, bq_cols], axis=1).astype(np.float32)  # [128,4]

    # ---- NEFF C: matmul + ELU (staging stays on device)
    nc_c = _built_c()
    fn_c, in_c, out_c, av_c, _ = _pjrt_fn(nc_c)
    host_c = {
        "wt": np.concatenate([np.ascontiguousarray(W.T)] * NCORES, axis=0),
        "aff": np.concatenate([aff] * NCORES, axis=0),
    }
    args_c = []
    for nm in in_c:
        if nm == "xt":
            args_c.append(outs_a["xt"])
        else:
            args_c.append(jax.device_put(host_c[nm], sharding))
    outs_c = fn_c(*args_c, *[jax.device_put(z, sharding) for z in _zeros_for(av_c)])
    y_bf = np.asarray(outs_c[out_c.index("yt")])
    with jax.default_device(cpu):
        # yt blocked [core, t, q, p, j, c] -> y[row t*1024+c*8+j, fout q*128+p]
        yt6 = jnp.asarray(y_bf).reshape(NCORES, T, 2, P, RT, P)
        y = np.asarray(
            jnp.transpose(yt6, (0, 1, 5, 4, 2, 3))
            .astype(jnp.float32)
            .reshape(N_TOTAL, F)
        )
    return np.ascontiguousarray(y)


if __name__ == "__main__":
    nca = build_a()
    ncc = build_c()
    print("built OK")
